# revision 25
# baseline (speedup 1.0000x reference)
"""Expert-mixture (top-1 MoE) Trainium2 kernel, expert-parallel across 8 cores
with 2-segment load balancing.

Strategy:
  - Host computes the router (x @ Wr + br, argmax) and dispatches tokens.
  - Each core gets TWO weight slots: a big "B" segment (SEG_B tokens, its
    primary expert) and a small "A" segment (SEG_A tokens, a remainder chunk
    of possibly another expert).  With counts ~2048 +- 230, every expert's
    bulk fits one B slot and the spill chunks (<= 8 x SEG_A total for the
    deterministic seed-0 reference) fill the A slots, cutting the uniform
    per-core capacity from max-count (2197) to SEG_A+SEG_B = 2081.  Overflow
    beyond the slots is computed on host (correct, just slower).
  - Core: hT = relu(W1seg.T @ xT + b1seg) ; outT = W2seg.T @ hT per block,
    blocks [512, 512, 512, 481 | 64] with the segment boundary between them.
  - Host scatters each slot's rows back into the full [B, C] output and adds
    b2[expert] (bias add commutes with the gather).

Schedule notes (derived from NTFF traces):
  - ~7us fixed NEFF preamble gates all engines; the DMA fabric ramps from
    ~8.7us to ~370GB/s.  Head-critical w1b-h1 / xt0 ride per-k slice DMAs
    in consumption order — the Tile framework gives per-transfer cumulative
    semaphore waits, so GEMM1 starts ~10.5us, paced by the ramping fabric
    (k0's weights go as two 128KB halves so the first matmul's dep clears
    the fabric sooner).  Consolidating chunks into one wide transfer makes
    mid-k consumers wait for ALL of it — keep the splits.
  - Block 0 uses GS=8 (its k-loop is fabric-paced; GS=4 would consume w1
    chunks 2x faster than they land); later blocks GS=4 so two 4-bank PSUM
    halves ping-pong and group g+1's matmuls overlap group g's relu drain.
  - Everything not needed in the first two blocks (xt2/xt3, w1a, A-consts)
    is deferred into the compute loop to keep the head window uncontended.
  - PE warmup matmuls bridge preamble -> data-ready (HAM clock ramps
    1.2 -> 2.37GHz after ~6us of sustained PE activity).
  - Slots much below ~128 tokens are rate-limited by LDWEIGHTS (~50ns per
    matmul floor), so finer balancing than (64, 2017) does not pay.

The builder is exec'd from a string with a fixed pseudo-filename so the
emitted BIR is byte-identical no matter where this file lives — keeping the
NEFF compile cache warm across directories.
"""

import numpy as np

import concourse.mybir as mybir
import concourse.tile as tile
from concourse import bacc
from concourse.bass_utils import run_bass_kernel_spmd

B, D, H, E, C = 16384, 1024, 2048, 8, 3
N_CORES = 8
P = 128
KD = D // P    # 8 contraction chunks for GEMM1
MH = H // P    # 16 H chunks
TB = 512       # token block (matmul moving dim)
SEG_A = 64     # small per-core slot (remainder chunks)
SEG_B = 2017   # big per-core slot (primary expert bulk)
CAP = SEG_A + SEG_B   # 2081 uniform per-core token capacity

MM_DTYPE = mybir.dt.bfloat16  # PE compute dtype
WARMUP_MMS = 17   # dummy PE matmuls to lift the HAM clock gate early;
                  # sized to overlap data-arrival jitter (~10.5-11us) so
                  # the gate never sees an activity gap and re-arms late

_nc_cache: dict = {}

_BUILDER_SRC = '''
def _build(cap, reps, mm_dtype, warmup_mms, mybir, tile, bacc):
    B, D, H, E, C = 16384, 1024, 2048, 8, 3
    N_CORES, P = 8, 128
    KD, MH, TB = D // P, H // P, 512
    SEG_A, SEG_B = 64, 2017
    assert cap == SEG_A + SEG_B
    HH = H // 2

    # blocks: (tok_off, tok_len, segment)
    blocks = [(0, 512, "b"), (512, 512, "b"), (1024, 512, "b"),
              (1536, SEG_B - 1536, "b"), (SEG_B, SEG_A, "a")]

    nc = bacc.Bacc("TRN2", target_bir_lowering=False, debug=False,
                   num_devices=N_CORES)
    f32 = mybir.dt.float32
    xt = nc.dram_tensor("xt", [D, cap], mm_dtype, kind="ExternalInput").ap()
    w1b = nc.dram_tensor("w1b", [D, H], mm_dtype, kind="ExternalInput").ap()
    w1a = nc.dram_tensor("w1a", [D, H], mm_dtype, kind="ExternalInput").ap()
    b1tb = nc.dram_tensor("b1tb", [P, MH], f32, kind="ExternalInput").ap()
    b1ta = nc.dram_tensor("b1ta", [P, MH], f32, kind="ExternalInput").ap()
    w2tb = nc.dram_tensor("w2tb", [P, MH * C], mm_dtype,
                          kind="ExternalInput").ap()
    w2ta = nc.dram_tensor("w2ta", [P, MH * C], mm_dtype,
                          kind="ExternalInput").ap()
    ot = nc.dram_tensor("ot", [C, cap], f32, kind="ExternalOutput").ap()

    xt3 = xt.rearrange("(k p) t -> p k t", p=P)
    w1b3 = w1b.rearrange("(k p) h -> p k h", p=P)
    w1a3 = w1a.rearrange("(k p) h -> p k h", p=P)

    with tile.TileContext(nc) as tc:
        with (
            tc.tile_pool(name="w1p", bufs=1) as w1p,
            tc.tile_pool(name="xtp", bufs=1) as xtp,
            tc.tile_pool(name="cst", bufs=1) as cst,
            tc.tile_pool(name="htp", bufs=1) as htp,
            tc.tile_pool(name="o2p", bufs=1) as o2p,
            tc.tile_pool(name="ps", bufs=1, space="PSUM") as psp,
        ):
            def body(_iv=None):
                # PE warmup: dummy matmuls during the ~2us head DMA so the
                # HAM clock gate starts ramping before the first real matmul.
                if warmup_mms:
                    wu = cst.tile([P, 256], mm_dtype, tag="wu", name="wu")
                    nc.gpsimd.memset(wu[:], 0.0)
                    wups = psp.tile([P, 256], f32, tag="ps", bufs=8,
                                    name="wups")
                    for _ in range(warmup_mms):
                        nc.tensor.matmul(wups[:], wu[:, :P], wu[:],
                                         start=True, stop=True)

                # ---- head-critical loads: per-k slice DMAs in consumption
                #      order (consumers get per-transfer cumulative waits).
                #      xt0-k0 rides SP first so the first matmul's two deps
                #      are the first two fabric transfers. ----
                t0off, t0sz = blocks[0][0], blocks[0][1]
                w1bh1_f = w1p.tile([P, KD * HH], mm_dtype, tag="w1bh1",
                                   name="w1bh1")
                w1bh1_3 = w1bh1_f[:].rearrange("p (k h) -> p k h", k=KD)
                xt0_f = xtp.tile([P, KD * t0sz], mm_dtype, tag="xt0",
                                 name="xt0")
                xt0_3 = xt0_f[:].rearrange("p (k t) -> p k t", k=KD)

                # k0's weights go as two 128KB halves: the first matmul
                # only needs m0-3, and a smaller first transfer clears the
                # still-ramping fabric sooner.
                nc.sync.dma_start(w1bh1_3[:, 0:1, 0:HH // 2],
                                  w1b3[:, 0:1, 0:HH // 2])
                nc.sync.dma_start(w1bh1_3[:, 0:1, HH // 2:HH],
                                  w1b3[:, 0:1, HH // 2:HH])
                for k in range(1, KD):
                    nc.sync.dma_start(w1bh1_3[:, k:k + 1, :],
                                      w1b3[:, k:k + 1, 0:HH])
                for k in range(KD):
                    nc.scalar.dma_start(xt0_3[:, k:k + 1, :],
                                        xt3[:, k:k + 1, t0off:t0off + t0sz])

                def w1bh1(k):
                    return w1bh1_f[:, k * HH:(k + 1) * HH]

                def xt0(k):
                    return xt0_f[:, k * t0sz:(k + 1) * t0sz]

                # ---- bulk loads (head-adjacent, still split for early
                #      partial consumption) ----
                # SP after h1: b1b, w2b (tiny, needed at first relu/GEMM2),
                # then w1b h2 per-k in consumption order.
                b1b_sb = cst.tile([P, MH], f32, tag="b1b", name="b1b_sb")
                nc.sync.dma_start(b1b_sb[:], b1tb[:])
                w2b_sb = cst.tile([P, MH * C], mm_dtype, tag="w2b",
                                  name="w2b_sb")
                nc.sync.dma_start(w2b_sb[:], w2tb[:])
                w1bh2_f = w1p.tile([P, KD * HH], mm_dtype, tag="w1bh2",
                                   name="w1bh2")
                w1bh2_3 = w1bh2_f[:].rearrange("p (k h) -> p k h", k=KD)
                for k in range(KD):
                    nc.sync.dma_start(w1bh2_3[:, k:k + 1, :],
                                      w1b3[:, k:k + 1, HH:H])

                def w1bh2(k):
                    return w1bh2_f[:, k * HH:(k + 1) * HH]

                # Activation: xt block 1 wide (needed only after block 0).
                t1off, t1sz = blocks[1][0], blocks[1][1]
                xt1_f = xtp.tile([P, KD * t1sz], mm_dtype, tag="xt1",
                                 name="xt1")
                nc.scalar.dma_start(
                    xt1_f[:].rearrange("p (k t) -> p k t", k=KD),
                    xt3[:, :, t1off:t1off + t1sz])

                def xt1(k):
                    return xt1_f[:, k * t1sz:(k + 1) * t1sz]

                # deferred into the compute loop (keep the head window
                # uncontended): xt2+xt3 after block0, w1a after block1,
                # A-segment consts after block2 — all on SP.
                xt_tiles = {}
                for t in (2, 3):
                    tsz = blocks[t][1]
                    xt_tiles[t] = xtp.tile([P, KD * tsz], mm_dtype,
                                           tag="xt%d" % t, name="xt_%d" % t)
                xta = xtp.tile([P, KD * SEG_A], mm_dtype, tag="xta",
                               name="xt_a")
                b1a_sb = cst.tile([P, MH], f32, tag="b1a", name="b1a_sb")
                w2a_sb = cst.tile([P, MH * C], mm_dtype, tag="w2a",
                                  name="w2a_sb")
                w1a_sb = w1p.tile([P, KD * H], mm_dtype, tag="w1a",
                                  name="w1a_sb")

                def fire_deferred(t):
                    if t == 0:
                        for tt in (2, 3):
                            toff, tsz = blocks[tt][0], blocks[tt][1]
                            nc.sync.dma_start(
                                xt_tiles[tt][:].rearrange(
                                    "p (k t) -> p k t", k=KD),
                                xt3[:, :, toff:toff + tsz])
                    elif t == 1:
                        nc.sync.dma_start(
                            w1a_sb[:].rearrange("p (k h) -> p k h", k=KD),
                            w1a3[:, :, :])
                    elif t == 2:
                        nc.sync.dma_start(
                            xta[:].rearrange("p (k t) -> p k t", k=KD),
                            xt3[:, :, SEG_B:SEG_B + SEG_A])
                        nc.sync.dma_start(b1a_sb[:], b1ta[:])
                        nc.sync.dma_start(w2a_sb[:], w2ta[:])

                def w1chunk(seg, k, m):
                    """lhsT [P, P] for contraction chunk k, output chunk m."""
                    if seg == "a":
                        return w1a_sb[:, k * H + m * P:k * H + (m + 1) * P]
                    if m < 8:
                        return w1bh1(k)[:, m * P:(m + 1) * P]
                    return w1bh2(k)[:, (m - 8) * P:(m - 7) * P]

                def xtchunk(t, k, tsz):
                    if t == 0:
                        return xt0(k)[:, :tsz]
                    if t == 1:
                        return xt1(k)[:, :tsz]
                    if t == 4:
                        return xta[:, k * SEG_A:k * SEG_A + tsz]
                    return xt_tiles[t][:, k * tsz:(k + 1) * tsz]

                o2_sb = o2p.tile([C, cap], f32, tag="o2", name="o2_sb")

                for t, (toff, tsz, seg) in enumerate(blocks):
                    b1_sb = b1b_sb if seg == "b" else b1a_sb
                    w2_sb = w2b_sb if seg == "b" else w2a_sb
                    # GS=4: two 4-bank PSUM halves ping-pong, so group g+1's
                    # matmuls overlap group g's relu drain (GS=8 used all 8
                    # banks and cost a ~430ns bubble per group transition).
                    # Block 0 stays GS=8: its k-loop is paced against the
                    # ramping DMA fabric, and GS=4 would consume w1 chunks
                    # 2x faster than they can land.
                    if t == 0:
                        GS = 8
                    else:
                        GS = 2 if tsz < 256 else 4
                    ht_tiles = []
                    for g in range(MH // GS):
                        ps_g = []
                        for mi in range(GS):
                            ps1 = psp.tile([P, TB], f32, tag="ps", bufs=8,
                                           name="ps1_%d_%d_%d" % (t, g, mi))
                            ps_g.append(ps1)
                        for k in range(KD):
                            for mi in range(GS):
                                m = g * GS + mi
                                nc.tensor.matmul(
                                    ps_g[mi][:, :tsz],
                                    w1chunk(seg, k, m),
                                    xtchunk(t, k, tsz),
                                    start=(k == 0),
                                    stop=(k == KD - 1),
                                )
                        for mi in range(GS):
                            m = g * GS + mi
                            ht = htp.tile([P, TB], mm_dtype, tag="ht%d" % m,
                                          name="ht_%d_%d" % (t, m))
                            # alternate relu chunks between Activation and
                            # DVE so the chain drains 2x faster.  (Moving
                            # ALL relus to DVE to skip the act-table
                            # preamble load was tried and is WORSE: the
                            # table queue loads regardless and the relu
                            # chain serializes GEMM2 — +1.3us.)
                            if mi % 2 == 0:
                                nc.scalar.activation(
                                    ht[:, :tsz], ps_g[mi][:, :tsz],
                                    mybir.ActivationFunctionType.Relu,
                                    bias=b1_sb[:, m:m + 1],
                                )
                            else:
                                nc.vector.tensor_scalar(
                                    ht[:, :tsz], ps_g[mi][:, :tsz],
                                    b1_sb[:, m:m + 1], 0.0,
                                    op0=mybir.AluOpType.add,
                                    op1=mybir.AluOpType.max,
                                )
                            ht_tiles.append(ht)

                    ps2 = psp.tile([C, TB], f32, tag="ps", bufs=8,
                                   name="ps2_%d" % t)
                    for m in range(MH):
                        nc.tensor.matmul(
                            ps2[:, :tsz],
                            w2_sb[:, m * C:(m + 1) * C],
                            ht_tiles[m][:, :tsz],
                            start=(m == 0),
                            stop=(m == MH - 1),
                        )
                    nc.vector.tensor_copy(o2_sb[:, toff:toff + tsz],
                                          ps2[:, :tsz])
                    # NOTE: single_packet=True on this store was tried and
                    # costs +27us (slow descriptor path) — never use it.
                    nc.sync.dma_start(ot[:, toff:toff + tsz],
                                      o2_sb[:, toff:toff + tsz])
                    fire_deferred(t)

            if reps == 1:
                body()
            else:
                hints = (mybir.EngineType.PE, mybir.EngineType.SP,
                         mybir.EngineType.Activation, mybir.EngineType.DVE)
                with tc.For_i(0, reps, 1, hint_engines=hints) as iv:
                    body(iv)

    nc.compile()
    return nc
'''

_builder_ns: dict = {}
exec(compile(_BUILDER_SRC, "<moe_builder>", "exec"), _builder_ns)


def build_nc(cap: int, reps: int = 1, mm_dtype=None):
    """Build + compile the SPMD program. reps>1 wraps the body in a device
    loop (for steady-state timing)."""
    if mm_dtype is None:
        mm_dtype = MM_DTYPE
    return _builder_ns["_build"](cap, reps, mm_dtype, WARMUP_MMS,
                                 mybir, tile, bacc)


def _get_nc(cap: int):
    key = (cap, MM_DTYPE)
    if key not in _nc_cache:
        _nc_cache[key] = build_nc(cap)
    return _nc_cache[key]


def _expert_mlp_host(xr, W1e, b1e, W2e, b2e):
    h = np.maximum(xr.astype(np.float32) @ W1e + b1e, 0.0)
    return h @ W2e + b2e


def _to_mm(a: np.ndarray) -> np.ndarray:
    """Convert f32 host data to the matmul storage dtype."""
    if MM_DTYPE == mybir.dt.float32r:
        b = np.ascontiguousarray(a, dtype=np.float32).copy().view(np.uint32)
        b += 0x00000FFF + ((b >> 13) & 1)
        b &= np.uint32(0xFFFFE000)
        return b.view(np.float32)
    if MM_DTYPE == mybir.dt.bfloat16:
        import ml_dtypes
        return np.ascontiguousarray(a).astype(ml_dtypes.bfloat16)
    return np.ascontiguousarray(a, dtype=np.float32)


def _plan_slots(idx):
    """Assign tokens to per-core (B, A) slots.

    Returns (b_tok, b_exp, a_tok, a_exp, overflow): per-core token index
    arrays + expert ids, and a list of (expert, token_idx_array) overflow
    pieces for the host fallback.
    """
    b_tok = [idx[e][:SEG_B] for e in range(E)]
    b_exp = list(range(E))
    rem = []
    for e in range(E):
        r = idx[e][SEG_B:]
        for off in range(0, len(r), SEG_A):
            rem.append((e, r[off:off + SEG_A]))
    a_tok = [np.empty(0, dtype=np.int64)] * N_CORES
    a_exp = [0] * N_CORES
    overflow = []
    for i, (e, chunk) in enumerate(rem):
        if i < N_CORES:
            a_tok[i] = chunk
            a_exp[i] = e
        else:
            overflow.append((e, chunk))
    return b_tok, b_exp, a_tok, a_exp, overflow


def make_in_maps(x, W1, b1, W2, idx, cap=CAP):
    assert cap == CAP
    b_tok, b_exp, a_tok, a_exp, _ = _plan_slots(idx)
    in_maps = []
    for i in range(N_CORES):
        xtc = np.zeros((D, CAP), dtype=np.float32)
        nb = len(b_tok[i])
        xtc[:, :nb] = x[b_tok[i]].T
        na = len(a_tok[i])
        if na:
            xtc[:, SEG_B:SEG_B + na] = x[a_tok[i]].T
        eb, ea = b_exp[i], a_exp[i]
        in_maps.append({
            "xt": _to_mm(xtc),
            "w1b": _to_mm(W1[eb]),
            "w1a": _to_mm(W1[ea]),
            "b1tb": np.ascontiguousarray(b1[eb].reshape(MH, P).T),
            "b1ta": np.ascontiguousarray(b1[ea].reshape(MH, P).T),
            "w2tb": _to_mm(W2[eb].reshape(MH, P, C).transpose(1, 0, 2)
                           .reshape(P, MH * C)),
            "w2ta": _to_mm(W2[ea].reshape(MH, P, C).transpose(1, 0, 2)
                           .reshape(P, MH * C)),
        })
    return in_maps


def kernel(x, Wr, br, W1, b1, W2, b2):
    x = np.asarray(x, dtype=np.float32)
    Wr = np.asarray(Wr, dtype=np.float32)
    br = np.asarray(br, dtype=np.float32)
    W1 = np.asarray(W1, dtype=np.float32)
    b1 = np.asarray(b1, dtype=np.float32)
    W2 = np.asarray(W2, dtype=np.float32)
    b2 = np.asarray(b2, dtype=np.float32)

    # Router on host: decides the sharding. CPU jax so near-tie argmax
    # rounds exactly like the reference; numpy fallback otherwise.
    try:
        import jax
        import jax.numpy as jnp
        with jax.default_device(jax.devices("cpu")[0]):
            logits = np.asarray(jnp.asarray(x) @ jnp.asarray(Wr)
                                + jnp.asarray(br))
    except Exception:
        logits = x @ Wr + br
    topics = np.argmax(logits, axis=1)

    idx = [np.flatnonzero(topics == e) for e in range(E)]
    b_tok, b_exp, a_tok, a_exp, overflow = _plan_slots(idx)
    in_maps = make_in_maps(x, W1, b1, W2, idx, CAP)
    nc = _get_nc(CAP)
    res = run_bass_kernel_spmd(nc, in_maps, core_ids=list(range(N_CORES)))

    out = np.empty((B, C), dtype=np.float32)
    for i in range(N_CORES):
        otv = res.results[i]["ot"]
        nb = len(b_tok[i])
        out[b_tok[i]] = otv[:, :nb].T + b2[b_exp[i]]
        na = len(a_tok[i])
        if na:
            out[a_tok[i]] = otv[:, SEG_B:SEG_B + na].T + b2[a_exp[i]]
    for e, chunk in overflow:
        out[chunk] = _expert_mlp_host(x[chunk], W1[e], b1[e], W2[e], b2[e])
    return out


# revision 26
# speedup vs baseline: 1.0054x; 1.0054x over previous
"""Expert-mixture (top-1 MoE) Trainium2 kernel, expert-parallel across 8 cores
with 2-segment load balancing.

Strategy:
  - Host computes the router (x @ Wr + br, argmax) and dispatches tokens.
  - Each core gets TWO weight slots: a big "B" segment (SEG_B tokens, its
    primary expert) and a small "A" segment (SEG_A tokens, a remainder chunk
    of possibly another expert).  With counts ~2048 +- 230, every expert's
    bulk fits one B slot and the spill chunks (<= 8 x SEG_A total for the
    deterministic seed-0 reference) fill the A slots, cutting the uniform
    per-core capacity from max-count (2197) to SEG_A+SEG_B = 2081.  Overflow
    beyond the slots is computed on host (correct, just slower).
  - Core: hT = relu(W1seg.T @ xT + b1seg) ; outT = W2seg.T @ hT per block,
    blocks [512, 512, 512, 481 | 64] with the segment boundary between them.
  - Host scatters each slot's rows back into the full [B, C] output and adds
    b2[expert] (bias add commutes with the gather).

Schedule notes (derived from NTFF traces):
  - ~7us fixed NEFF preamble gates all engines; the DMA fabric ramps from
    ~8.7us to ~370GB/s.  Head-critical w1b-h1 / xt0 ride per-k slice DMAs
    in consumption order — the Tile framework gives per-transfer cumulative
    semaphore waits, so GEMM1 starts ~10.5us, paced by the ramping fabric
    (k0's weights go as two 128KB halves so the first matmul's dep clears
    the fabric sooner).  Consolidating chunks into one wide transfer makes
    mid-k consumers wait for ALL of it — keep the splits.
  - Block 0 uses GS=8 (its k-loop is fabric-paced; GS=4 would consume w1
    chunks 2x faster than they land); later blocks GS=4 so two 4-bank PSUM
    halves ping-pong and group g+1's matmuls overlap group g's relu drain.
  - Everything not needed in the first two blocks (xt2/xt3, w1a, A-consts)
    is deferred into the compute loop to keep the head window uncontended.
  - PE warmup matmuls bridge preamble -> data-ready (HAM clock ramps
    1.2 -> 2.37GHz after ~6us of sustained PE activity).
  - Slots much below ~128 tokens are rate-limited by LDWEIGHTS (~50ns per
    matmul floor), so finer balancing than (64, 2017) does not pay.

The builder is exec'd from a string with a fixed pseudo-filename so the
emitted BIR is byte-identical no matter where this file lives — keeping the
NEFF compile cache warm across directories.
"""

import numpy as np

import concourse.mybir as mybir
import concourse.tile as tile
from concourse import bacc
from concourse.bass_utils import run_bass_kernel_spmd

B, D, H, E, C = 16384, 1024, 2048, 8, 3
N_CORES = 8
P = 128
KD = D // P    # 8 contraction chunks for GEMM1
MH = H // P    # 16 H chunks
TB = 512       # token block (matmul moving dim)
SEG_A = 64     # small per-core slot (remainder chunks)
SEG_B = 2017   # big per-core slot (primary expert bulk)
CAP = SEG_A + SEG_B   # 2081 uniform per-core token capacity

MM_DTYPE = mybir.dt.bfloat16  # PE compute dtype
WARMUP_MMS = 14   # dummy PE matmuls to lift the HAM clock gate early.
                  # 14 is the measured EV optimum: 12 leaves a pre-data
                  # idle gap, 17 (bridging data-arrival jitter) costs
                  # +0.4-1us and does NOT remove the slow-run outliers.

_nc_cache: dict = {}

_BUILDER_SRC = '''
def _build(cap, reps, mm_dtype, warmup_mms, mybir, tile, bacc):
    B, D, H, E, C = 16384, 1024, 2048, 8, 3
    N_CORES, P = 8, 128
    KD, MH, TB = D // P, H // P, 512
    SEG_A, SEG_B = 64, 2017
    assert cap == SEG_A + SEG_B
    HH = H // 2

    # blocks: (tok_off, tok_len, segment)
    blocks = [(0, 512, "b"), (512, 512, "b"), (1024, 512, "b"),
              (1536, SEG_B - 1536, "b"), (SEG_B, SEG_A, "a")]

    nc = bacc.Bacc("TRN2", target_bir_lowering=False, debug=False,
                   num_devices=N_CORES)
    f32 = mybir.dt.float32
    xt = nc.dram_tensor("xt", [D, cap], mm_dtype, kind="ExternalInput").ap()
    w1b = nc.dram_tensor("w1b", [D, H], mm_dtype, kind="ExternalInput").ap()
    w1a = nc.dram_tensor("w1a", [D, H], mm_dtype, kind="ExternalInput").ap()
    b1tb = nc.dram_tensor("b1tb", [P, MH], f32, kind="ExternalInput").ap()
    b1ta = nc.dram_tensor("b1ta", [P, MH], f32, kind="ExternalInput").ap()
    w2tb = nc.dram_tensor("w2tb", [P, MH * C], mm_dtype,
                          kind="ExternalInput").ap()
    w2ta = nc.dram_tensor("w2ta", [P, MH * C], mm_dtype,
                          kind="ExternalInput").ap()
    ot = nc.dram_tensor("ot", [C, cap], f32, kind="ExternalOutput").ap()

    xt3 = xt.rearrange("(k p) t -> p k t", p=P)
    w1b3 = w1b.rearrange("(k p) h -> p k h", p=P)
    w1a3 = w1a.rearrange("(k p) h -> p k h", p=P)

    with tile.TileContext(nc) as tc:
        with (
            tc.tile_pool(name="w1p", bufs=1) as w1p,
            tc.tile_pool(name="xtp", bufs=1) as xtp,
            tc.tile_pool(name="cst", bufs=1) as cst,
            tc.tile_pool(name="htp", bufs=1) as htp,
            tc.tile_pool(name="o2p", bufs=1) as o2p,
            tc.tile_pool(name="ps", bufs=1, space="PSUM") as psp,
        ):
            def body(_iv=None):
                # PE warmup: dummy matmuls during the ~2us head DMA so the
                # HAM clock gate starts ramping before the first real matmul.
                if warmup_mms:
                    wu = cst.tile([P, 256], mm_dtype, tag="wu", name="wu")
                    nc.gpsimd.memset(wu[:], 0.0)
                    wups = psp.tile([P, 256], f32, tag="ps", bufs=8,
                                    name="wups")
                    for _ in range(warmup_mms):
                        nc.tensor.matmul(wups[:], wu[:, :P], wu[:],
                                         start=True, stop=True)

                # ---- head-critical loads: per-k slice DMAs in consumption
                #      order (consumers get per-transfer cumulative waits).
                #      xt0-k0 rides SP first so the first matmul's two deps
                #      are the first two fabric transfers. ----
                t0off, t0sz = blocks[0][0], blocks[0][1]
                w1bh1_f = w1p.tile([P, KD * HH], mm_dtype, tag="w1bh1",
                                   name="w1bh1")
                w1bh1_3 = w1bh1_f[:].rearrange("p (k h) -> p k h", k=KD)
                xt0_f = xtp.tile([P, KD * t0sz], mm_dtype, tag="xt0",
                                 name="xt0")
                xt0_3 = xt0_f[:].rearrange("p (k t) -> p k t", k=KD)

                # k0's weights go as two 128KB halves: the first matmul
                # only needs m0-3, and a smaller first transfer clears the
                # still-ramping fabric sooner.
                nc.sync.dma_start(w1bh1_3[:, 0:1, 0:HH // 2],
                                  w1b3[:, 0:1, 0:HH // 2])
                nc.sync.dma_start(w1bh1_3[:, 0:1, HH // 2:HH],
                                  w1b3[:, 0:1, HH // 2:HH])
                for k in range(1, KD):
                    nc.sync.dma_start(w1bh1_3[:, k:k + 1, :],
                                      w1b3[:, k:k + 1, 0:HH])
                for k in range(KD):
                    nc.scalar.dma_start(xt0_3[:, k:k + 1, :],
                                        xt3[:, k:k + 1, t0off:t0off + t0sz])

                def w1bh1(k):
                    return w1bh1_f[:, k * HH:(k + 1) * HH]

                def xt0(k):
                    return xt0_f[:, k * t0sz:(k + 1) * t0sz]

                # ---- bulk loads (head-adjacent, still split for early
                #      partial consumption) ----
                # SP after h1: b1b, w2b (tiny, needed at first relu/GEMM2),
                # then w1b h2 per-k in consumption order.
                b1b_sb = cst.tile([P, MH], f32, tag="b1b", name="b1b_sb")
                nc.sync.dma_start(b1b_sb[:], b1tb[:])
                w2b_sb = cst.tile([P, MH * C], mm_dtype, tag="w2b",
                                  name="w2b_sb")
                nc.sync.dma_start(w2b_sb[:], w2tb[:])
                w1bh2_f = w1p.tile([P, KD * HH], mm_dtype, tag="w1bh2",
                                   name="w1bh2")
                w1bh2_3 = w1bh2_f[:].rearrange("p (k h) -> p k h", k=KD)
                for k in range(KD):
                    nc.sync.dma_start(w1bh2_3[:, k:k + 1, :],
                                      w1b3[:, k:k + 1, HH:H])

                def w1bh2(k):
                    return w1bh2_f[:, k * HH:(k + 1) * HH]

                # Activation: xt block 1 wide (needed only after block 0).
                t1off, t1sz = blocks[1][0], blocks[1][1]
                xt1_f = xtp.tile([P, KD * t1sz], mm_dtype, tag="xt1",
                                 name="xt1")
                nc.scalar.dma_start(
                    xt1_f[:].rearrange("p (k t) -> p k t", k=KD),
                    xt3[:, :, t1off:t1off + t1sz])

                def xt1(k):
                    return xt1_f[:, k * t1sz:(k + 1) * t1sz]

                # deferred into the compute loop (keep the head window
                # uncontended): xt2+xt3 after block0, w1a after block1,
                # A-segment consts after block2 — all on SP.
                xt_tiles = {}
                for t in (2, 3):
                    tsz = blocks[t][1]
                    xt_tiles[t] = xtp.tile([P, KD * tsz], mm_dtype,
                                           tag="xt%d" % t, name="xt_%d" % t)
                xta = xtp.tile([P, KD * SEG_A], mm_dtype, tag="xta",
                               name="xt_a")
                b1a_sb = cst.tile([P, MH], f32, tag="b1a", name="b1a_sb")
                w2a_sb = cst.tile([P, MH * C], mm_dtype, tag="w2a",
                                  name="w2a_sb")
                w1a_sb = w1p.tile([P, KD * H], mm_dtype, tag="w1a",
                                  name="w1a_sb")

                def fire_deferred(t):
                    if t == 0:
                        for tt in (2, 3):
                            toff, tsz = blocks[tt][0], blocks[tt][1]
                            nc.sync.dma_start(
                                xt_tiles[tt][:].rearrange(
                                    "p (k t) -> p k t", k=KD),
                                xt3[:, :, toff:toff + tsz])
                    elif t == 1:
                        nc.sync.dma_start(
                            w1a_sb[:].rearrange("p (k h) -> p k h", k=KD),
                            w1a3[:, :, :])
                    elif t == 2:
                        nc.sync.dma_start(
                            xta[:].rearrange("p (k t) -> p k t", k=KD),
                            xt3[:, :, SEG_B:SEG_B + SEG_A])
                        nc.sync.dma_start(b1a_sb[:], b1ta[:])
                        nc.sync.dma_start(w2a_sb[:], w2ta[:])

                def w1chunk(seg, k, m):
                    """lhsT [P, P] for contraction chunk k, output chunk m."""
                    if seg == "a":
                        return w1a_sb[:, k * H + m * P:k * H + (m + 1) * P]
                    if m < 8:
                        return w1bh1(k)[:, m * P:(m + 1) * P]
                    return w1bh2(k)[:, (m - 8) * P:(m - 7) * P]

                def xtchunk(t, k, tsz):
                    if t == 0:
                        return xt0(k)[:, :tsz]
                    if t == 1:
                        return xt1(k)[:, :tsz]
                    if t == 4:
                        return xta[:, k * SEG_A:k * SEG_A + tsz]
                    return xt_tiles[t][:, k * tsz:(k + 1) * tsz]

                o2_sb = o2p.tile([C, cap], f32, tag="o2", name="o2_sb")

                for t, (toff, tsz, seg) in enumerate(blocks):
                    b1_sb = b1b_sb if seg == "b" else b1a_sb
                    w2_sb = w2b_sb if seg == "b" else w2a_sb
                    # GS=4: two 4-bank PSUM halves ping-pong, so group g+1's
                    # matmuls overlap group g's relu drain (GS=8 used all 8
                    # banks and cost a ~430ns bubble per group transition).
                    # Block 0 stays GS=8: its k-loop is paced against the
                    # ramping DMA fabric, and GS=4 would consume w1 chunks
                    # 2x faster than they can land.
                    if t == 0:
                        GS = 8
                    else:
                        GS = 2 if tsz < 256 else 4
                    ht_tiles = []
                    for g in range(MH // GS):
                        ps_g = []
                        for mi in range(GS):
                            ps1 = psp.tile([P, TB], f32, tag="ps", bufs=8,
                                           name="ps1_%d_%d_%d" % (t, g, mi))
                            ps_g.append(ps1)
                        for k in range(KD):
                            for mi in range(GS):
                                m = g * GS + mi
                                nc.tensor.matmul(
                                    ps_g[mi][:, :tsz],
                                    w1chunk(seg, k, m),
                                    xtchunk(t, k, tsz),
                                    start=(k == 0),
                                    stop=(k == KD - 1),
                                )
                        for mi in range(GS):
                            m = g * GS + mi
                            ht = htp.tile([P, TB], mm_dtype, tag="ht%d" % m,
                                          name="ht_%d_%d" % (t, m))
                            # alternate relu chunks between Activation and
                            # DVE so the chain drains 2x faster.  (Moving
                            # ALL relus to DVE to skip the act-table
                            # preamble load was tried and is WORSE: the
                            # table queue loads regardless and the relu
                            # chain serializes GEMM2 — +1.3us.)
                            if mi % 2 == 0:
                                nc.scalar.activation(
                                    ht[:, :tsz], ps_g[mi][:, :tsz],
                                    mybir.ActivationFunctionType.Relu,
                                    bias=b1_sb[:, m:m + 1],
                                )
                            else:
                                nc.vector.tensor_scalar(
                                    ht[:, :tsz], ps_g[mi][:, :tsz],
                                    b1_sb[:, m:m + 1], 0.0,
                                    op0=mybir.AluOpType.add,
                                    op1=mybir.AluOpType.max,
                                )
                            ht_tiles.append(ht)

                    ps2 = psp.tile([C, TB], f32, tag="ps", bufs=8,
                                   name="ps2_%d" % t)
                    for m in range(MH):
                        nc.tensor.matmul(
                            ps2[:, :tsz],
                            w2_sb[:, m * C:(m + 1) * C],
                            ht_tiles[m][:, :tsz],
                            start=(m == 0),
                            stop=(m == MH - 1),
                        )
                    nc.vector.tensor_copy(o2_sb[:, toff:toff + tsz],
                                          ps2[:, :tsz])
                    # NOTE: single_packet=True on this store was tried and
                    # costs +27us (slow descriptor path) — never use it.
                    nc.sync.dma_start(ot[:, toff:toff + tsz],
                                      o2_sb[:, toff:toff + tsz])
                    fire_deferred(t)

            if reps == 1:
                body()
            else:
                hints = (mybir.EngineType.PE, mybir.EngineType.SP,
                         mybir.EngineType.Activation, mybir.EngineType.DVE)
                with tc.For_i(0, reps, 1, hint_engines=hints) as iv:
                    body(iv)

    nc.compile()
    return nc
'''

_builder_ns: dict = {}
exec(compile(_BUILDER_SRC, "<moe_builder>", "exec"), _builder_ns)


def build_nc(cap: int, reps: int = 1, mm_dtype=None):
    """Build + compile the SPMD program. reps>1 wraps the body in a device
    loop (for steady-state timing)."""
    if mm_dtype is None:
        mm_dtype = MM_DTYPE
    return _builder_ns["_build"](cap, reps, mm_dtype, WARMUP_MMS,
                                 mybir, tile, bacc)


def _get_nc(cap: int):
    key = (cap, MM_DTYPE)
    if key not in _nc_cache:
        _nc_cache[key] = build_nc(cap)
    return _nc_cache[key]


def _expert_mlp_host(xr, W1e, b1e, W2e, b2e):
    h = np.maximum(xr.astype(np.float32) @ W1e + b1e, 0.0)
    return h @ W2e + b2e


def _to_mm(a: np.ndarray) -> np.ndarray:
    """Convert f32 host data to the matmul storage dtype."""
    if MM_DTYPE == mybir.dt.float32r:
        b = np.ascontiguousarray(a, dtype=np.float32).copy().view(np.uint32)
        b += 0x00000FFF + ((b >> 13) & 1)
        b &= np.uint32(0xFFFFE000)
        return b.view(np.float32)
    if MM_DTYPE == mybir.dt.bfloat16:
        import ml_dtypes
        return np.ascontiguousarray(a).astype(ml_dtypes.bfloat16)
    return np.ascontiguousarray(a, dtype=np.float32)


def _plan_slots(idx):
    """Assign tokens to per-core (B, A) slots.

    Returns (b_tok, b_exp, a_tok, a_exp, overflow): per-core token index
    arrays + expert ids, and a list of (expert, token_idx_array) overflow
    pieces for the host fallback.
    """
    b_tok = [idx[e][:SEG_B] for e in range(E)]
    b_exp = list(range(E))
    rem = []
    for e in range(E):
        r = idx[e][SEG_B:]
        for off in range(0, len(r), SEG_A):
            rem.append((e, r[off:off + SEG_A]))
    a_tok = [np.empty(0, dtype=np.int64)] * N_CORES
    a_exp = [0] * N_CORES
    overflow = []
    for i, (e, chunk) in enumerate(rem):
        if i < N_CORES:
            a_tok[i] = chunk
            a_exp[i] = e
        else:
            overflow.append((e, chunk))
    return b_tok, b_exp, a_tok, a_exp, overflow


def make_in_maps(x, W1, b1, W2, idx, cap=CAP):
    assert cap == CAP
    b_tok, b_exp, a_tok, a_exp, _ = _plan_slots(idx)
    in_maps = []
    for i in range(N_CORES):
        xtc = np.zeros((D, CAP), dtype=np.float32)
        nb = len(b_tok[i])
        xtc[:, :nb] = x[b_tok[i]].T
        na = len(a_tok[i])
        if na:
            xtc[:, SEG_B:SEG_B + na] = x[a_tok[i]].T
        eb, ea = b_exp[i], a_exp[i]
        in_maps.append({
            "xt": _to_mm(xtc),
            "w1b": _to_mm(W1[eb]),
            "w1a": _to_mm(W1[ea]),
            "b1tb": np.ascontiguousarray(b1[eb].reshape(MH, P).T),
            "b1ta": np.ascontiguousarray(b1[ea].reshape(MH, P).T),
            "w2tb": _to_mm(W2[eb].reshape(MH, P, C).transpose(1, 0, 2)
                           .reshape(P, MH * C)),
            "w2ta": _to_mm(W2[ea].reshape(MH, P, C).transpose(1, 0, 2)
                           .reshape(P, MH * C)),
        })
    return in_maps


def kernel(x, Wr, br, W1, b1, W2, b2):
    x = np.asarray(x, dtype=np.float32)
    Wr = np.asarray(Wr, dtype=np.float32)
    br = np.asarray(br, dtype=np.float32)
    W1 = np.asarray(W1, dtype=np.float32)
    b1 = np.asarray(b1, dtype=np.float32)
    W2 = np.asarray(W2, dtype=np.float32)
    b2 = np.asarray(b2, dtype=np.float32)

    # Router on host: decides the sharding. CPU jax so near-tie argmax
    # rounds exactly like the reference; numpy fallback otherwise.
    try:
        import jax
        import jax.numpy as jnp
        with jax.default_device(jax.devices("cpu")[0]):
            logits = np.asarray(jnp.asarray(x) @ jnp.asarray(Wr)
                                + jnp.asarray(br))
    except Exception:
        logits = x @ Wr + br
    topics = np.argmax(logits, axis=1)

    idx = [np.flatnonzero(topics == e) for e in range(E)]
    b_tok, b_exp, a_tok, a_exp, overflow = _plan_slots(idx)
    in_maps = make_in_maps(x, W1, b1, W2, idx, CAP)
    nc = _get_nc(CAP)
    res = run_bass_kernel_spmd(nc, in_maps, core_ids=list(range(N_CORES)))

    out = np.empty((B, C), dtype=np.float32)
    for i in range(N_CORES):
        otv = res.results[i]["ot"]
        nb = len(b_tok[i])
        out[b_tok[i]] = otv[:, :nb].T + b2[b_exp[i]]
        na = len(a_tok[i])
        if na:
            out[a_tok[i]] = otv[:, SEG_B:SEG_B + na].T + b2[a_exp[i]]
    for e, chunk in overflow:
        out[chunk] = _expert_mlp_host(x[chunk], W1[e], b1[e], W2[e], b2[e])
    return out
